# revision 1
# baseline (speedup 1.0000x reference)
"""CrissCrossAttention kernel for TRN2 — 8-core data-parallel over batch.

Per core (one batch element, x:[512, 9216] f32):
  q = Wq@x + bq   [64, 9216]   (bf16 matmuls, fp32 accum)
  k = Wk@x + bk   [64, 9216]
  v = Wv@x + bv   [512, 9216], stored transposed as vT[u, h, c]
  eH[h,w,g] = <q[:,h,w], k[:,g,w]>  (diag h==g masked to -inf, fused
              into the PSUM accumulation as a -30000*I rank-96 update)
  eW[h,w,u] = <q[:,h,w], k[:,h,u]>
  att = softmax over concat(g,u)  -> out = gamma*(aH@v_col + aW@v_row) + x

Softmax uses a constant shift (exp(e - SHIFT)) instead of a row max —
valid here because e = <q,k> over 64 channels with unit-variance q,k is
bounded well inside fp32 exp range. Normalization (1/Z, and gamma) is
applied once at the end via a rank-1 PE broadcast.

Phase 2+3 run twice over channel halves (ct 0-1, then ct 2-3) so the
attention accumulator A fits in SBUF next to vT; attention weights are
recomputed per half (cheap relative to the U matmuls).
"""

import numpy as np

import concourse.bass as bass
import concourse.bacc as bacc
import concourse.tile as tile
import concourse.mybir as mybir
from concourse.bass_utils import run_bass_kernel_spmd

F32 = mybir.dt.float32
BF = mybir.dt.bfloat16
AF = mybir.ActivationFunctionType
ALU = mybir.AluOpType

B = 8
C = 512
CQ = 64
HH = 96
S = HH * HH  # 9216
KT = 4  # c_in tiles of 128
CT = 4  # c_out tiles of 128
CH1 = 384  # phase-1 spatial chunk (4 rows of 96)
NCH1 = S // CH1  # 24
CH3 = 384  # phase-3 spatial chunk (4 rows of 96)
NCH3 = S // CH3  # 24
SHIFT = 20.0
DIAG_NEG = -30000.0


def _build_nc():
    nc = bacc.Bacc("TRN2", target_bir_lowering=False, debug=False)

    x_d = nc.dram_tensor("x", [C, S], F32, kind="ExternalInput")
    wq_d = nc.dram_tensor("wq", [CQ, C], F32, kind="ExternalInput")
    bq_d = nc.dram_tensor("bq", [CQ, 1], F32, kind="ExternalInput")
    wk_d = nc.dram_tensor("wk", [CQ, C], F32, kind="ExternalInput")
    bk_d = nc.dram_tensor("bk", [CQ, 1], F32, kind="ExternalInput")
    wv_d = nc.dram_tensor("wv", [C, C], F32, kind="ExternalInput")
    bv_d = nc.dram_tensor("bv", [1, C], F32, kind="ExternalInput")
    g_d = nc.dram_tensor("gamma", [1, 1], F32, kind="ExternalInput")
    out_d = nc.dram_tensor("out", [C, S], F32, kind="ExternalOutput")

    with tile.TileContext(nc, pool_alloc_mode="queue") as tc:
        _body(tc, x_d, wq_d, bq_d, wk_d, bk_d, wv_d, bv_d, g_d, out_d)
    nc.compile()
    return nc


def _body(tc, x_d, wq_d, bq_d, wk_d, bk_d, wv_d, bv_d, g_d, out_d):
    nc = tc.nc

    consts_cm = tc.tile_pool(name="consts", bufs=1)
    consts = consts_cm.__enter__()
    zpool_cm = tc.tile_pool(name="zpool", bufs=1)
    zpool = zpool_cm.__enter__()

    # ---- constants ----
    ones128 = consts.tile([128, 128], F32, tag="ones128")
    nc.vector.memset(ones128[:], 1.0)
    ident = consts.tile([128, 128], F32, tag="ident")
    nc.gpsimd.affine_select(
        ident[:], ones128[:], [[-1, 128]], ALU.is_equal, 0.0,
        base=0, channel_multiplier=1,
    )
    negt = consts.tile([HH, HH], F32, tag="negt")
    nc.vector.memset(negt[:], DIAG_NEG)
    mask_f32 = consts.tile([HH, HH], F32, tag="mask_f32")
    nc.gpsimd.affine_select(
        mask_f32[:], negt[:], [[-1, HH]], ALU.is_equal, 0.0,
        base=0, channel_multiplier=1,
    )
    ones_col = consts.tile([HH, 1], BF, tag="ones_col")
    nc.vector.memset(ones_col[:], 1.0)
    ones_row = consts.tile([1, 128], BF, tag="ones_row")
    nc.vector.memset(ones_row[:], 1.0)
    g_sb = consts.tile([1, 1], F32, tag="g_sb")
    nc.sync.dma_start(g_sb[:], g_d[:])
    g_bf = consts.tile([1, 1], BF, tag="g_bf")
    nc.vector.tensor_copy(g_bf[:], g_sb[:])
    bq_sb = consts.tile([CQ, 1], F32, tag="bq_sb")
    nc.sync.dma_start(bq_sb[:], bq_d[:])
    bk_sb = consts.tile([CQ, 1], F32, tag="bk_sb")
    nc.sync.dma_start(bk_sb[:], bk_d[:])
    shiftb = consts.tile([HH, 1], F32, tag="shiftb")
    nc.vector.memset(shiftb[:], -SHIFT)

    # Z accumulators + final gamma/Z row (tiny, live to the end)
    zh = zpool.tile([HH, HH], F32, tag="zh")
    zw = zpool.tile([HH, HH], F32, tag="zw")
    zgp = zpool.tile([HH, HH], BF, tag="zgp")

    qkv_cm = tc.tile_pool(name="qkv", bufs=1)
    qkv = qkv_cm.__enter__()
    qsb = qkv.tile([CQ, S], BF, tag="qsb")
    ksb = qkv.tile([CQ, S], BF, tag="ksb")
    # v transposed: vT[u, h, c] — row tile h = vT[:, h, :], col tile w = vT[w, :, :]
    vT = qkv.tile([HH, HH, C], BF, tag="vT")

    # ---------------- phase 0: weights -----------------
    with tc.tile_pool(name="wts", bufs=1) as wts:
        wqT = [wts.tile([128, CQ], BF, tag=f"wqT{kt}", name=f"wqT{kt}")
               for kt in range(KT)]
        wkT = [wts.tile([128, CQ], BF, tag=f"wkT{kt}", name=f"wkT{kt}")
               for kt in range(KT)]
        wvT = [wts.tile([128, C], BF, tag=f"wvT{kt}", name=f"wvT{kt}")
               for kt in range(KT)]
        bvB = wts.tile([HH, C], F32, tag="bvB")

        with (
            tc.tile_pool(name="wload", bufs=1) as wload,
            tc.tile_pool(name="wpsum", bufs=2, space="PSUM") as wpsum,
        ):
            wq_sb = wload.tile([CQ, C], F32, tag="wq_sb")
            nc.sync.dma_start(wq_sb[:], wq_d[:])
            wk_sb = wload.tile([CQ, C], F32, tag="wk_sb")
            nc.sync.dma_start(wk_sb[:], wk_d[:])
            wv_sb = [wload.tile([128, C], F32, tag=f"wv_sb{ct}", name=f"wv_sb{ct}")
                     for ct in range(CT)]
            for ct in range(CT):
                nc.sync.dma_start(wv_sb[ct][:], wv_d[ct * 128:(ct + 1) * 128, :])
            bv_row = wload.tile([1, C], F32, tag="bv_row")
            nc.sync.dma_start(bv_row[:], bv_d[:])
            bv_row_bf = wload.tile([1, C], BF, tag="bv_row_bf")
            nc.vector.tensor_copy(bv_row_bf[:], bv_row[:])

            for kt in range(KT):
                tp = wpsum.tile([128, CQ], F32, tag="tpq")
                nc.tensor.transpose(
                    tp[:], wq_sb[:, kt * 128:(kt + 1) * 128], ident[:CQ, :CQ]
                )
                nc.scalar.copy(wqT[kt][:], tp[:])
                tp2 = wpsum.tile([128, CQ], F32, tag="tpq")
                nc.tensor.transpose(
                    tp2[:], wk_sb[:, kt * 128:(kt + 1) * 128], ident[:CQ, :CQ]
                )
                nc.scalar.copy(wkT[kt][:], tp2[:])
                for ct in range(CT):
                    tpv = wpsum.tile([128, 128], F32, tag="tpv")
                    nc.tensor.transpose(
                        tpv[:], wv_sb[ct][:, kt * 128:(kt + 1) * 128], ident[:]
                    )
                    nc.scalar.copy(wvT[kt][:, ct * 128:(ct + 1) * 128], tpv[:])

            # bvB[p, c] = bv[c] for all p (rank-1 PE broadcast)
            bvp = wpsum.tile([HH, C], F32, tag="bvp")
            nc.tensor.matmul(
                bvp[:], ones_row[:, :HH], bv_row_bf[:], start=True, stop=True
            )
            nc.scalar.copy(bvB[:], bvp[:])

        # x viewed as [p, kt, s]: partition p of k-tile kt
        x4 = x_d.rearrange("(kt p) s -> p kt s", p=128)
        # ---------------- phase 1: projections -----------------
        with (
            tc.tile_pool(name="p1", bufs=1) as p1,
            tc.tile_pool(name="p1ps", bufs=1, space="PSUM") as p1ps,
        ):
            for ci in range(NCH1):
                sl = slice(ci * CH1, (ci + 1) * CH1)
                xf = p1.tile([128, KT, CH1], F32, tag="xf", bufs=2)
                nc.sync.dma_start(xf[:], x4[:, :, sl])
                xb = p1.tile([128, KT, CH1], BF, tag="xb", bufs=2)
                nc.scalar.copy(xb[:], xf[:])

                qp = p1ps.tile([CQ, CH1], F32, tag="qkps", bufs=2)
                for kt in range(KT):
                    nc.tensor.matmul(
                        qp[:], wqT[kt][:], xb[:, kt, :],
                        start=(kt == 0), stop=(kt == KT - 1),
                    )
                nc.vector.tensor_scalar(qsb[:, sl], qp[:], bq_sb[:], None, ALU.add)
                kp = p1ps.tile([CQ, CH1], F32, tag="qkps", bufs=2)
                for kt in range(KT):
                    nc.tensor.matmul(
                        kp[:], wkT[kt][:], xb[:, kt, :],
                        start=(kt == 0), stop=(kt == KT - 1),
                    )
                nc.vector.tensor_scalar(ksb[:, sl], kp[:], bk_sb[:], None, ALU.add)

                # v rows, transposed: psum[u, c] for each spatial row h
                for j in range(CH1 // HH):
                    h = ci * (CH1 // HH) + j
                    vp = p1ps.tile([HH, C], F32, tag="vps", bufs=4)
                    for kt in range(KT):
                        nc.tensor.matmul(
                            vp[:], xb[:, kt, j * HH:(j + 1) * HH], wvT[kt][:],
                            start=(kt == 0), stop=(kt == KT - 1),
                        )
                    eng = nc.vector if j % 2 == 0 else nc.scalar
                    if eng is nc.vector:
                        eng.tensor_tensor(vT[:, h, :], vp[:], bvB[:], ALU.add)
                    else:
                        # ACT: copy+bias not fusable; bv contribution via copy
                        # then POOL add would double ops — route adds DVE/ACT
                        nc.vector.tensor_tensor(vT[:, h, :], vp[:], bvB[:], ALU.add)

    # ---------------- phase 2 + 3, per channel half -----------------
    q3 = qsb.rearrange("c (h w) -> c h w", w=HH)
    k3 = ksb.rearrange("c (h w) -> c h w", w=HH)

    out4 = out_d.rearrange("(kt p) s -> p kt s", p=128)
    x4b = x_d.rearrange("(kt p) s -> p kt s", p=128)

    for half in range(2):
        cts = (2 * half, 2 * half + 1)
        csl = slice(cts[0] * 128, (cts[1] + 1) * 128)  # 256 channels

        apool_cm = tc.tile_pool(name=f"apool{half}", bufs=1)
        apool = apool_cm.__enter__()
        A = [apool.tile([128, S], BF, tag=f"A{ct}", name=f"A{ct}") for ct in cts]
        A3 = [a.rearrange("c (h w) -> c h w", w=HH) for a in A]

        with (
            tc.tile_pool(name=f"p2_{half}", bufs=1) as p2,
            tc.tile_pool(name=f"p2ps_{half}", bufs=1, space="PSUM") as p2ps,
        ):
            # --- 2a: H-side (column attention): writes A (and zh in half 0)
            def emit_e_col(w):
                ep = p2ps.tile([HH, HH], F32, tag="eps", bufs=2)
                nc.tensor.matmul(
                    ep[:], k3[:, :, w], q3[:, :, w], start=True, stop=True
                )
                nc.vector.tensor_tensor(ep[:], ep[:], mask_f32[:], ALU.add)
                return ep

            def emit_vt_col(w):
                vt = p2.tile([HH, 256], BF, tag="vt", bufs=3)
                nc.sync.dma_start(vt[:], vT[w:w + 1, :, csl])
                return vt

            ep_next = emit_e_col(0)
            vt_next = emit_vt_col(0)
            for w in range(HH):
                ep, vt = ep_next, vt_next
                if w + 1 < HH:
                    ep_next = emit_e_col(w + 1)
                    vt_next = emit_vt_col(w + 1)
                pt = p2.tile([HH, HH], BF, tag="pt", bufs=2)
                nc.scalar.activation(pt[:], ep[:], AF.Exp, bias=shiftb[:])
                up = p2ps.tile([128, 2, HH], F32, tag="ups", bufs=2)
                for i in range(2):
                    nc.tensor.matmul(
                        up[:, i, :], vt[:, i * 128:(i + 1) * 128], pt[:],
                        start=True, stop=True,
                    )
                if half == 0:
                    zp = p2ps.tile([HH, 1], F32, tag="zps", bufs=2)
                    nc.tensor.matmul(
                        zp[:], pt[:], ones_col[:], start=True, stop=True
                    )
                    nc.vector.tensor_copy(zh[:, w:w + 1], zp[:])
                if w % 2 == 0:
                    nc.vector.tensor_copy(A3[0][:, :, w], up[:, 0, :])
                    nc.scalar.copy(A3[1][:, :, w], up[:, 1, :])
                else:
                    nc.scalar.copy(A3[0][:, :, w], up[:, 0, :])
                    nc.vector.tensor_copy(A3[1][:, :, w], up[:, 1, :])

            # --- 2b: W-side (row attention): accumulates into A (+zw half 0)
            def emit_e_row(h):
                sl = slice(h * HH, (h + 1) * HH)
                ep = p2ps.tile([HH, HH], F32, tag="eps", bufs=2)
                nc.tensor.matmul(
                    ep[:], ksb[:, sl], qsb[:, sl], start=True, stop=True
                )
                return ep

            ep_next = emit_e_row(0)
            for h in range(HH):
                sl = slice(h * HH, (h + 1) * HH)
                ep = ep_next
                if h + 1 < HH:
                    ep_next = emit_e_row(h + 1)
                pt = p2.tile([HH, HH], BF, tag="pt", bufs=2)
                nc.scalar.activation(pt[:], ep[:], AF.Exp, bias=shiftb[:])
                up = p2ps.tile([128, 2, HH], F32, tag="ups", bufs=2)
                for i in range(2):
                    nc.tensor.matmul(
                        up[:, i, :],
                        vT[:, h, cts[i] * 128:(cts[i] + 1) * 128], pt[:],
                        start=True, stop=True,
                    )
                if half == 0:
                    zp = p2ps.tile([HH, 1], F32, tag="zps", bufs=2)
                    nc.tensor.matmul(
                        zp[:], pt[:], ones_col[:], start=True, stop=True
                    )
                    nc.vector.tensor_copy(zw[:, h:h + 1], zp[:])
                # A[:, h-row] += up: psum reads on DVE; second ct bounced
                # through ACT copy + POOL add (POOL cannot read PSUM)
                nc.vector.tensor_tensor(
                    A[0][:, sl], A[0][:, sl], up[:, 0, :], ALU.add
                )
                tmp = p2.tile([128, HH], BF, tag="tmp", bufs=2)
                nc.scalar.copy(tmp[:], up[:, 1, :])
                nc.gpsimd.tensor_tensor(A[1][:, sl], A[1][:, sl], tmp[:], ALU.add)

            if half == 0:
                # --- 2c: Ztot -> gamma/Z as bf16 row chunks ---
                zwt = p2ps.tile([HH, HH], F32, tag="zwt", bufs=1)
                nc.tensor.transpose(zwt[:], zw[:], ident[:HH, :HH])
                zsum = p2.tile([HH, HH], F32, tag="zsum")
                nc.vector.tensor_tensor(zsum[:], zh[:], zwt[:], ALU.add)
                zrec = p2.tile([HH, HH], F32, tag="zrec")
                nc.vector.reciprocal(zrec[:], zsum[:])
                gps = p2ps.tile([HH, 1], F32, tag="gps", bufs=1)
                nc.tensor.matmul(
                    gps[:], ones_row[:, :HH], g_bf[:], start=True, stop=True
                )
                gbc = p2.tile([HH, 1], F32, tag="gbc")
                nc.scalar.copy(gbc[:], gps[:])
                nc.vector.tensor_scalar(zgp[:], zrec[:], gbc[:], None, ALU.mult)

        # ---------------- phase 3 (this half): scale + residual ----------
        with (
            tc.tile_pool(name=f"p3_{half}", bufs=1) as p3,
            tc.tile_pool(name=f"p3ps_{half}", bufs=1, space="PSUM") as p3ps,
        ):
            for ci in range(NCH3):
                sl = slice(ci * CH3, (ci + 1) * CH3)
                xr = p3.tile([128, 2, CH3], F32, tag="xr", bufs=2)
                nc.sync.dma_start(xr[:], x4b[:, 2 * half:2 * half + 2, sl])
                zst = p3.tile([1, CH3], BF, tag="zst", bufs=2)
                nc.sync.dma_start(zst[:], zgp[4 * ci:4 * ci + 4, :])
                zb = p3ps.tile([128, CH3], F32, tag="zbps", bufs=2)
                nc.tensor.matmul(
                    zb[:], ones_row[:], zst[:], start=True, stop=True
                )
                for i in range(2):
                    tmp3 = p3.tile([128, CH3], F32, tag="tmp3", bufs=3)
                    nc.vector.tensor_tensor(
                        tmp3[:], A[i][:, sl], zb[:], ALU.mult
                    )
                    e = nc.vector if i == 0 else nc.gpsimd
                    e.tensor_tensor(xr[:, i, :], xr[:, i, :], tmp3[:], ALU.add)
                nc.sync.dma_start(out4[:, 2 * half:2 * half + 2, sl], xr[:])

        apool_cm.__exit__(None, None, None)

    qkv_cm.__exit__(None, None, None)
    zpool_cm.__exit__(None, None, None)
    consts_cm.__exit__(None, None, None)


_NC_CACHE = None


def _get_nc():
    global _NC_CACHE
    if _NC_CACHE is None:
        _NC_CACHE = _build_nc()
    return _NC_CACHE


def _in_maps(x, Wq, bq, Wk, bk, Wv, bv, gamma):
    x = np.ascontiguousarray(np.asarray(x, dtype=np.float32))
    shared = {
        "wq": np.ascontiguousarray(np.asarray(Wq, np.float32)),
        "bq": np.ascontiguousarray(np.asarray(bq, np.float32).reshape(CQ, 1)),
        "wk": np.ascontiguousarray(np.asarray(Wk, np.float32)),
        "bk": np.ascontiguousarray(np.asarray(bk, np.float32).reshape(CQ, 1)),
        "wv": np.ascontiguousarray(np.asarray(Wv, np.float32)),
        "bv": np.ascontiguousarray(np.asarray(bv, np.float32).reshape(1, C)),
        "gamma": np.ascontiguousarray(np.asarray(gamma, np.float32).reshape(1, 1)),
    }
    maps = []
    for b in range(B):
        m = dict(shared)
        m["x"] = np.ascontiguousarray(x[min(b, x.shape[0] - 1)].reshape(C, S))
        maps.append(m)
    return maps


def run(inputs, trace=False):
    nc = _get_nc()
    maps = _in_maps(**inputs)
    res = run_bass_kernel_spmd(nc, maps, core_ids=list(range(B)), trace=trace)
    out = np.stack(
        [res.results[b]["out"].reshape(C, HH, HH) for b in range(B)], axis=0
    )
    return out.astype(np.float32), res


def kernel(x, Wq, bq, Wk, bk, Wv, bv, gamma):
    out, _ = run(dict(x=x, Wq=Wq, bq=bq, Wk=Wk, bk=bk, Wv=Wv, bv=bv, gamma=gamma))
    return out

